# revision 74
# baseline (speedup 1.0000x reference)
"""MoE layer (E=8, top-2) on 8 NeuronCores via Bass/Tile.

Strategy: exact expert-parallel token dispatch.
  Core e holds expert e's weights and receives exactly the tokens whose
  top-2 routing includes expert e (host computes the routing assignment --
  a pure data-placement decision -- and gathers those tokens, padded to a
  static capacity CAP).  The device re-computes the gate scores for its
  tokens against the full (replicated, permuted) gate and derives the
  renormalized top-2 combine weight of its own expert as
      w_local = sigmoid(2*s_local - m1 - m2)
  (m1/m2 = top-2 score values).  This equals the reference's renormalized
  softmax weight whenever the local expert is within the device's top-2 and
  degrades smoothly (no cliff) on rank-boundary ties, so it is robust to
  fp32-vs-bf16 ranking differences vs the host dispatch.
  Each core then runs its expert's dense MLP over its CAP tokens (bf16
  matmuls, fp32 PSUM accumulation), scales by w_local, and the host
  scatter-adds the two expert contributions per token back together.

  Activations keep hidden dim on partitions (transposed) so all matmuls
  consume natural-layout weights.  Tokens are processed in two column
  groups so every PSUM accumulator tile fits one 2KB PSUM bank; the
  groups share accumulator banks (sequential reuse).

  Scheduling (guided by the TimelineSim cost model):
  - all DRAM loads ride one HWDGE queue in first-need order, w1/w2 split
    into pieces matched to the PE consumption rate (transfers serialize at
    full bandwidth anyway, so global order is what matters);
  - the router's sigmoids run before the first FFN activation and group 0
    computes silu(h) as h*sigmoid(h) (b1==0 here), because Silu and
    Sigmoid live in different ACT function tables and each mid-stream
    table swap costs 1.3us;
  - mm2 trails mm1 by DEFER steps so the PE never stalls on the ACT/DVE
    activation chain or the early w2 DMA;
  - the last group's activations are pre-scaled by the combine weight so
    its epilogue is a bare PSUM->SBUF copy split across ACT and DVE,
    shortening the drain tail;
  - b1==0 / b2==0 variants are compiled on demand from the actual inputs
    (general fallbacks keep kernel() correct for arbitrary operands).
"""

import numpy as np
import ml_dtypes

# Problem shapes (hardcoded per the task contract).
B, S, H, F, E = 2, 1024, 512, 2048, 8
T = B * S              # 2048 tokens
N_CORES = 8
CAP = 556              # per-expert token capacity (actual max is 554)
HC = H // 128          # 4
FC = F // 128          # 16
CGS = [288, 268]       # token column groups (each fits a PSUM bank)

_cache = {}


def _build_bass(b1_zero=True, b2_zero=True):
    import concourse.mybir as mybir
    import concourse.tile as tile
    from concourse import bacc

    f32 = mybir.dt.float32
    bf16 = mybir.dt.bfloat16

    nc = bacc.Bacc(None, target_bir_lowering=False, debug=False)
    with tile.TileContext(nc) as tc:
        with tc.tile_pool(name="dram", bufs=1, space="DRAM") as dram:
            xT_d = dram.tile([H, CAP], bf16, kind="ExternalInput", name="xT", uniquify=False)
            w1_d = dram.tile([H, F], bf16, kind="ExternalInput", name="w1", uniquify=False)
            w2_d = dram.tile([F, H], bf16, kind="ExternalInput", name="w2", uniquify=False)
            smf_d = dram.tile([128, FC + HC * E], f32, kind="ExternalInput", name="smf", uniquify=False)
            b2r_d = dram.tile([1, H], bf16, kind="ExternalInput", name="b2r", uniquify=False)
            outT_d = dram.tile([H, CAP], bf16, kind="ExternalOutput", name="outT", uniquify=False)
            _moe_body(nc, tc, mybir, xT_d, w1_d, w2_d, smf_d, b2r_d, outT_d, b1_zero, b2_zero)
    nc.compile()
    return nc


def _moe_body(nc, tc, mybir, xT_d, w1_d, w2_d, smf_d, b2r_d, outT_d, b1_zero=True, b2_zero=True):
    from concourse.masks import make_identity

    f32 = mybir.dt.float32
    bf16 = mybir.dt.bfloat16
    ALU = mybir.AluOpType
    ACTF = mybir.ActivationFunctionType
    AXIS = mybir.AxisListType

    col_of = [0]
    for tg in CGS[:-1]:
        col_of.append(col_of[-1] + tg)

    with (
        tc.tile_pool(name="constp", bufs=1) as constp,
        tc.tile_pool(name="xp", bufs=1) as xp,
        tc.tile_pool(name="wp", bufs=1) as wp,
        tc.tile_pool(name="actp", bufs=3) as actp,
        tc.tile_pool(name="rp", bufs=2) as rp,
        tc.tile_pool(name="php", bufs=2, space="PSUM") as php,
        tc.tile_pool(name="pop", bufs=1, space="PSUM") as pop,
        tc.tile_pool(name="pmp", bufs=2, space="PSUM") as pmp,
    ):
        # ---- constants & input loads (split across 4 DMA queues so the
        # first mm1 can start ~3us in and nothing arrives late) ----
        identity = constp.tile([128, 128], f32, name="identity")
        make_identity(nc, identity)
        onesb = constp.tile([1, 512], bf16, name="onesb")
        nc.vector.memset(onesb, 1.0)

        # gpsimd (SWDGE) queue: small tensors -- separate issue pipe from the
        # HWDGE queue, so these land early without consuming HWDGE slots.
        # b1t+wgb packed in one tensor so a single early DMA covers both.
        smf = xp.tile([128, FC + HC * E], f32, name="smf", tag="smf")
        nc.gpsimd.dma_start(out=smf, in_=smf_d[:, :])
        b1t = smf[:, 0:FC]
        wgb = xp.tile([128, HC * E], bf16, name="wgb", tag="wgb")
        nc.vector.tensor_copy(out=wgb, in_=smf[:, FC:FC + HC * E])
        b2r = None
        if not b2_zero:
            b2r = xp.tile([1, H], bf16, name="b2r", tag="b2r")
            nc.gpsimd.dma_start(out=b2r, in_=b2r_d[:, :])

        # All large loads on ONE queue, ordered by first-need time and split
        # into pieces sized to the PE's consumption rate.  DMA transfers
        # serialize at full bandwidth in hardware anyway (a single large copy
        # already spans all 16 engines), so a deliberate global order beats
        # spreading across queues.
        PIECES = [2, 2, 4, 4, 4]            # fc counts per w1/w2 piece
        pc_of = []                          # fc -> (piece, offset)
        for pi, nfc in enumerate(PIECES):
            pc_of += [(pi, k) for k in range(nfc)]
        xg = [None] * len(CGS)
        w1q = [None] * len(PIECES)
        w2q = [None] * len(PIECES)

        def load_xg(gi):
            tg = CGS[gi]
            xt = xp.tile([128, HC, tg], bf16, name=f"xs{gi}", tag=f"xs{gi}")
            nc.sync.dma_start(
                out=xt,
                in_=xT_d[:, col_of[gi]:col_of[gi] + tg].rearrange("(hc p) t -> p hc t", p=128))
            xg[gi] = xt

        def load_w1q(pi):
            f0 = sum(PIECES[:pi]) * 128
            nf = PIECES[pi] * 128
            wt = wp.tile([128, HC, nf], bf16, name=f"w1_{pi}", tag=f"w1_{pi}")
            nc.sync.dma_start(
                out=wt,
                in_=w1_d[:, f0:f0 + nf].rearrange("(hc p) f -> p hc f", p=128))
            w1q[pi] = wt

        def load_w2q(pi):
            f0 = sum(PIECES[:pi]) * 128
            wt = wp.tile([128, PIECES[pi], H], bf16, name=f"w2_{pi}", tag=f"w2_{pi}")
            nc.sync.dma_start(
                out=wt,
                in_=w2_d[f0:f0 + PIECES[pi] * 128, :].rearrange("(fc p) h -> p fc h", p=128))
            w2q[pi] = wt

        load_xg(0)
        load_w1q(0)
        load_w2q(0)
        load_w1q(1)
        load_w2q(1)
        for gi in range(1, len(CGS)):
            load_xg(gi)
        for pi in range(2, len(PIECES)):
            load_w1q(pi)
            load_w2q(pi)

        def w1sl(hc, fc):
            pi, k = pc_of[fc]
            return w1q[pi][:, hc, k * 128:k * 128 + 128]

        def w2sl(hc, fc):
            pi, k = pc_of[fc]
            return w2q[pi][:, k, hc * 128:(hc + 1) * 128]

        # ---- router (emitted mid-stream): scores -> local expert's
        # renormalized top-2 weight, w = sigmoid(2*s0 - m1 - m2).  Exact when
        # the local expert is in the device top-2 (invariant to which slot),
        # degrades smoothly on rank-boundary ties. ----
        comb_sb = []
        comb_bf = []
        chunks = []                          # (group, local col, n)
        for gi, tg in enumerate(CGS):
            for c in range(0, tg, 128):
                chunks.append((gi, c, min(128, tg - c)))
        wcols = xp.tile([128, len(chunks)], f32, name="wcols", tag="wcols")

        def emit_router_scores():
            # all sigmoids run BEFORE the first silu: Silu and Sigmoid live
            # in different ACT function tables, each swap costs 1.3us.  The
            # per-chunk max/mask chain alternates DVE / gpsimd so the last
            # sigmoid (which gates the Silu table load) lands early.
            for ci, (gi, c, n) in enumerate(chunks):
                eng = nc.vector
                tsl = slice(c, c + n)
                ps = pmp.tile([128, E], f32, name=f"ps{ci}", tag="pm")
                for hc in range(HC):
                    nc.tensor.matmul(
                        out=ps[0:n, :], lhsT=xg[gi][:, hc, tsl], rhs=wgb[:, hc * E:(hc + 1) * E],
                        start=(hc == 0), stop=(hc == HC - 1),
                    )
                s = rp.tile([128, E], f32, name=f"s{ci}", tag="s")
                eng.tensor_copy(out=s[0:n, :], in_=ps[0:n, :])
                m1 = rp.tile([128, 1], f32, name=f"m1{ci}", tag="m1")
                eng.tensor_reduce(out=m1[0:n, :], in_=s[0:n, :], axis=AXIS.X, op=ALU.max)
                is1 = rp.tile([128, E], f32, name=f"is1{ci}", tag="is1")
                eng.tensor_scalar(out=is1[0:n, :], in0=s[0:n, :], scalar1=m1[0:n, :], scalar2=None, op0=ALU.is_ge)
                s2 = rp.tile([128, E], f32, name=f"s2{ci}", tag="s2")
                eng.scalar_tensor_tensor(
                    out=s2[0:n, :], in0=is1[0:n, :], scalar=-1e30, in1=s[0:n, :], op0=ALU.mult, op1=ALU.add,
                )
                m2 = rp.tile([128, 1], f32, name=f"m2{ci}", tag="m2")
                eng.tensor_reduce(out=m2[0:n, :], in_=s2[0:n, :], axis=AXIS.X, op=ALU.max)
                # nm = -m1 - m2
                nm = rp.tile([128, 1], f32, name=f"nm{ci}", tag="nm")
                eng.scalar_tensor_tensor(
                    out=nm[0:n, :], in0=m1[0:n, :], scalar=-1.0, in1=m2[0:n, :], op0=ALU.mult, op1=ALU.subtract,
                )
                nc.scalar.activation(
                    out=wcols[0:n, ci:ci + 1], in_=s[0:n, 0:1], func=ACTF.Sigmoid,
                    bias=nm[0:n, :], scale=2.0,
                )

        combT = xp.tile([1, CAP], bf16, name="combT", tag="combT")

        def emit_router_transposes():
            # transpose each w column chunk -> [1, n] row at partition 0 of a
            # per-group psum row tile (no bank ping-pong between chunks),
            # then one copy per group into the [1, CAP] sbuf row
            for gi, tg in enumerate(CGS):
                pst = pmp.tile([1, tg], f32, name=f"pstg{gi}", tag="pm")
                for ci, (cgi, c, n) in enumerate(chunks):
                    if cgi != gi:
                        continue
                    nc.tensor.transpose(
                        out=pst[:, c:c + n], in_=wcols[0:n, ci:ci + 1],
                        identity=identity[0:n, 0:n])
                nc.vector.tensor_copy(
                    out=combT[:, col_of[gi]:col_of[gi] + tg], in_=pst)

        def emit_router_combine():
            # broadcast each group's w row across the 128 partitions (outer
            # product with ones), one matmul per group.  The last group also
            # gets a bf16 copy: its activations are scaled on the way INTO
            # mm2 (the weight is ready long before that group runs), so its
            # epilogue is a plain PSUM->SBUF copy that ACT and DVE can share.
            for gi, tg in enumerate(CGS):
                cw = pmp.tile([128, tg], f32, name=f"combW{gi}", tag="pm")
                nc.tensor.matmul(
                    out=cw, lhsT=onesb[:, 0:128], rhs=combT[:, col_of[gi]:col_of[gi] + tg],
                    start=True, stop=True,
                )
                cs = xp.tile([128, tg], f32, name=f"combWs{gi}", tag=f"combWs{gi}")
                nc.vector.tensor_copy(out=cs, in_=cw)
                comb_sb.append(cs)
                if gi == len(CGS) - 1:
                    cb = xp.tile([128, tg], bf16, name="combWb", tag="combWb")
                    nc.vector.tensor_copy(out=cb, in_=cw)
                    comb_bf.append(cb)

        # ---- output accumulators (banks shared between the two groups) ----
        out_ps = []
        for gi, tg in enumerate(CGS):
            out_ps.append([pop.tile([128, tg], f32, name=f"outp{gi}_{hc}", tag=f"outp_{hc}")
                           for hc in range(HC)])

        # ---- main FFN loop; mm2 deferred two steps so PE never stalls on
        # the ACT silu chain or the early w2 DMA ----
        osb = xp.tile([128, HC, CAP], bf16, name="osb", tag="osb")
        steps = [(gi, fc) for gi in range(len(CGS)) for fc in range(FC)]
        DEFER = 3
        pending = []
        emit_router_scores()

        def emit_mm2(item):
            gi_p, fc_p, asil_p = item
            tg_p = CGS[gi_p]
            csl = slice(col_of[gi_p], col_of[gi_p] + tg_p)
            last_g = gi_p == len(CGS) - 1
            for hc in range(HC):
                nc.tensor.matmul(
                    out=out_ps[gi_p][hc], lhsT=w2sl(hc, fc_p),
                    rhs=asil_p, start=(fc_p == 0), stop=(fc_p == FC - 1 and b2_zero),
                )
                if fc_p == FC - 1:
                    if not b2_zero:
                        # b2 contribution as a rank-1 update closing the
                        # accum group, then scale by the combine weight;
                        # per-hc interleave so the tail drains per lane
                        nc.tensor.matmul(
                            out=out_ps[gi_p][hc], lhsT=b2r[:, hc * 128:(hc + 1) * 128],
                            rhs=onesb[:, 0:tg_p], start=False, stop=True,
                        )
                    if last_g and b2_zero:
                        # pre-scaled group: epilogue is a plain copy, split
                        # ACT / DVE so lanes drain in parallel (only these
                        # two engines can read PSUM)
                        if hc % 2 == 0:
                            nc.scalar.activation(
                                out=osb[:, hc, csl], in_=out_ps[gi_p][hc],
                                func=ACTF.Copy, bias=0.0, scale=1.0)
                        else:
                            nc.vector.tensor_copy(out=osb[:, hc, csl], in_=out_ps[gi_p][hc])
                    else:
                        # must be DVE: only DVE/ACT can read PSUM
                        nc.vector.tensor_mul(osb[:, hc, csl], out_ps[gi_p][hc], comb_sb[gi_p])
                    if last_g and hc % 2 == 1:
                        # store in hc pairs so the first transfer starts
                        # before the last lanes finish scaling
                        nc.sync.dma_start(
                            out=outT_d[(hc - 1) * 128:(hc + 1) * 128, csl].rearrange(
                                "(q p) t -> p q t", p=128),
                            in_=osb[:, hc - 1:hc + 1, csl])
            if fc_p == FC - 1 and not last_g:
                # store this group's columns (overlaps next group's compute)
                nc.sync.dma_start(
                    out=outT_d[:, csl].rearrange("(hc p) t -> p hc t", p=128),
                    in_=osb[:, :, csl])

        for si, (gi, fc) in enumerate(steps):
            tg = CGS[gi]
            hps = php.tile([128, tg], f32, name=f"h{gi}_{fc}", tag="h")
            for hc in range(HC):
                nc.tensor.matmul(
                    out=hps, lhsT=w1sl(hc, fc), rhs=xg[gi][:, hc, :],
                    start=(hc == 0), stop=(hc == HC - 1),
                )
            # Group 0 computes silu(h) as h * sigmoid(h): Sigmoid shares the
            # ACT function table with the router's sigmoids (Silu does not),
            # so the start of the kernel needs no table reload (1.3us each).
            # Group 1 switches to real Silu -- the one swap hides in ACT idle
            # mid-stream, and its shorter hps chain (no DVE hop) keeps the
            # PSUM hps buffers recycling fast in the final group.
            asil = actp.tile([128, tg], bf16, name=f"as{gi}_{fc}", tag="asil", bufs=DEFER + 2)
            if gi == 0 and b1_zero:
                sg = actp.tile([128, tg], bf16, name=f"sg{gi}_{fc}", tag="sg", bufs=DEFER + 2)
                nc.scalar.activation(
                    out=sg, in_=hps, func=ACTF.Sigmoid,
                    bias=b1t[:, fc:fc + 1], scale=1.0,
                )
                nc.vector.tensor_mul(asil, hps, sg)
            else:
                nc.scalar.activation(
                    out=asil, in_=hps, func=ACTF.Silu,
                    bias=b1t[:, fc:fc + 1], scale=1.0,
                )
            if gi == len(CGS) - 1 and b2_zero:
                # scale the last group's activations going INTO mm2 so its
                # epilogue needs no combine multiply
                asc = actp.tile([128, tg], bf16, name=f"ac{gi}_{fc}", tag="asc", bufs=DEFER + 2)
                nc.vector.tensor_mul(asc, asil, comb_bf[0])
                asil = asc
            pending.append((gi, fc, asil))
            if len(pending) > DEFER:
                emit_mm2(pending.pop(0))
            # router transposes/combine drop in once the sigmoids are done
            # (they sit in PE program order, so not too early)
            if si == 5:
                emit_router_transposes()
            elif si == 8:
                emit_router_combine()
        for item in pending:
            emit_mm2(item)


def _get_nc(b1_zero=True, b2_zero=True):
    key = ("nc", b1_zero, b2_zero)
    if key not in _cache:
        _cache[key] = _build_bass(b1_zero, b2_zero)
    return _cache[key]


def _route(x2d, Wg):
    """Host-side routing assignment (data placement only): token index lists
    per expert, matching the reference's fp32 top-2."""
    scores = x2d @ Wg.T                              # [T, E] fp32
    top1 = scores.argmax(1)
    s2 = scores.copy()
    s2[np.arange(scores.shape[0]), top1] = -np.inf
    top2 = s2.argmax(1)
    return [np.where((top1 == e) | (top2 == e))[0] for e in range(E)]


def _make_in_maps(x, Wg, W1, b1, W2, b2):
    x2d = np.ascontiguousarray(x.reshape(T, H), dtype=np.float32)
    idx = _route(x2d, Wg)
    xb = x2d.astype(ml_dtypes.bfloat16)
    in_maps = []
    for e in range(N_CORES):
        sel = idx[e][:CAP]
        n_e = len(sel)
        xT = np.zeros((H, CAP), ml_dtypes.bfloat16)
        xT[:, :n_e] = xb[sel].T
        perm = [e] + [i for i in range(E) if i != e]
        wgT = Wg[perm].T.astype(ml_dtypes.bfloat16)          # [H, E]
        wgb = wgT.reshape(HC, 128, E).transpose(1, 0, 2).reshape(128, HC * E)
        w1c = np.ascontiguousarray(W1[e]).astype(ml_dtypes.bfloat16)
        w2c = np.ascontiguousarray(W2[e]).astype(ml_dtypes.bfloat16)
        smf = np.empty((128, FC + HC * E), np.float32)
        smf[:, 0:FC] = b1[e].reshape(FC, 128).T
        smf[:, FC:] = wgb.astype(np.float32)
        b2rc = np.ascontiguousarray(b2[e].reshape(1, H)).astype(ml_dtypes.bfloat16)
        in_maps.append({"xT": xT, "w1": w1c, "w2": w2c, "smf": smf, "b2r": b2rc})
    return in_maps, idx


def _combine(outs, idx, x2d=None, Wg=None, W1=None, b1=None, W2=None, b2=None):
    """Scatter-add each expert's outputs back to token order."""
    of = np.zeros((T, H), np.float32)
    for e in range(N_CORES):
        sel = idx[e][:CAP]
        of[sel] += outs[e][:, :len(sel)].T.astype(np.float32)
        if len(idx[e]) > CAP and x2d is not None:
            # capacity overflow fallback (never hit for the staged shapes'
            # routing balance; exact fp32 math for generality)
            rest = idx[e][CAP:]
            sc = x2d[rest] @ Wg.T
            m1v = sc.max(1)
            s2v = sc.copy()
            s2v[np.arange(len(rest)), sc.argmax(1)] = -np.inf
            m2v = s2v.max(1)
            w = 1.0 / (1.0 + np.exp(-(2 * sc[:, e] - m1v - m2v)))
            hpre = x2d[rest] @ W1[e] + b1[e]
            a = hpre / (1.0 + np.exp(-hpre))
            of[rest] += w[:, None] * (a @ W2[e] + b2[e])
    return of.reshape(B, S, H)


def kernel(x, Wg, W1, b1, W2, b2, _trace=False, _trace_kwargs=None):
    from concourse.bass_utils import run_bass_kernel_spmd

    nc = _get_nc(b1_zero=bool(np.all(np.asarray(b1) == 0)),
                 b2_zero=bool(np.all(np.asarray(b2) == 0)))
    x = np.asarray(x, np.float32)
    Wg = np.asarray(Wg, np.float32)
    W1 = np.asarray(W1, np.float32)
    b1 = np.asarray(b1, np.float32)
    W2 = np.asarray(W2, np.float32)
    b2 = np.asarray(b2, np.float32)
    in_maps, idx = _make_in_maps(x, Wg, W1, b1, W2, b2)
    kw = {}
    if _trace:
        kw.update(trace=True, **(_trace_kwargs or {}))
    res = run_bass_kernel_spmd(nc, in_maps, core_ids=list(range(N_CORES)), **kw)
    _cache["last_results"] = res
    outs = [r["outT"] for r in res.results]
    return _combine(outs, idx, x.reshape(T, H), Wg, W1, b1, W2, b2)


# revision 77
# speedup vs baseline: 2.6219x; 2.6219x over previous
"""MoE layer (E=8, top-2) on 8 NeuronCores via Bass/Tile.

Strategy: exact expert-parallel token dispatch.
  Core e holds expert e's weights and receives exactly the tokens whose
  top-2 routing includes expert e (host computes the routing assignment --
  a pure data-placement decision -- and gathers those tokens, padded to a
  static capacity CAP).  The device re-computes the gate scores for its
  tokens against the full (replicated, permuted) gate and derives the
  renormalized top-2 combine weight of its own expert as
      w_local = sigmoid(2*s_local - m1 - m2)
  (m1/m2 = top-2 score values).  This equals the reference's renormalized
  softmax weight whenever the local expert is within the device's top-2 and
  degrades smoothly (no cliff) on rank-boundary ties, so it is robust to
  fp32-vs-bf16 ranking differences vs the host dispatch.
  Each core then runs its expert's dense MLP over its CAP tokens (bf16
  matmuls, fp32 PSUM accumulation), scales by w_local, and the host
  scatter-adds the two expert contributions per token back together.

  Activations keep hidden dim on partitions (transposed) so all matmuls
  consume natural-layout weights.  Tokens are processed in two column
  groups so every PSUM accumulator tile fits one 2KB PSUM bank; the
  groups share accumulator banks (sequential reuse).

  Scheduling (guided by the TimelineSim cost model):
  - all DRAM loads ride one HWDGE queue in first-need order, w1/w2 split
    into pieces matched to the PE consumption rate (transfers serialize at
    full bandwidth anyway, so global order is what matters);
  - the router's sigmoids run before the first FFN activation and group 0
    computes silu(h) as h*sigmoid(h) (b1==0 here), because Silu and
    Sigmoid live in different ACT function tables and each mid-stream
    table swap costs 1.3us;
  - mm2 trails mm1 by DEFER steps so the PE never stalls on the ACT/DVE
    activation chain or the early w2 DMA;
  - the last group's activations are pre-scaled by the combine weight so
    its epilogue is a bare PSUM->SBUF copy split across ACT and DVE,
    shortening the drain tail;
  - b1==0 / b2==0 variants are compiled on demand from the actual inputs
    (general fallbacks keep kernel() correct for arbitrary operands).
"""

import numpy as np
import ml_dtypes

# Problem shapes (hardcoded per the task contract).
B, S, H, F, E = 2, 1024, 512, 2048, 8
T = B * S              # 2048 tokens
N_CORES = 8
CAP = 556              # per-expert token capacity (actual max is 554)
HC = H // 128          # 4
FC = F // 128          # 16
CGS = [288, 268]       # token column groups (each fits a PSUM bank)

_cache = {}


def _build_bass(b1_zero=True, b2_zero=True):
    import concourse.mybir as mybir
    import concourse.tile as tile
    from concourse import bacc

    f32 = mybir.dt.float32
    bf16 = mybir.dt.bfloat16

    nc = bacc.Bacc(None, target_bir_lowering=False, debug=False)
    with tile.TileContext(nc) as tc:
        with tc.tile_pool(name="dram", bufs=1, space="DRAM") as dram:
            xT_d = dram.tile([H, CAP], bf16, kind="ExternalInput", name="xT", uniquify=False)
            w1_d = dram.tile([H, F], bf16, kind="ExternalInput", name="w1", uniquify=False)
            w2_d = dram.tile([F, H], bf16, kind="ExternalInput", name="w2", uniquify=False)
            smf_d = dram.tile([128, FC + HC * E], f32, kind="ExternalInput", name="smf", uniquify=False)
            b2r_d = dram.tile([1, H], bf16, kind="ExternalInput", name="b2r", uniquify=False)
            ind_d = dram.tile([8, 8 * 128], bf16, kind="ExternalInput", name="ind6", uniquify=False)
            outT_d = dram.tile([H, CAP], bf16, kind="ExternalOutput", name="outT", uniquify=False)
            _moe_body(nc, tc, mybir, xT_d, w1_d, w2_d, smf_d, b2r_d, ind_d, outT_d, b1_zero, b2_zero)
    nc.compile()
    return nc


def _moe_body(nc, tc, mybir, xT_d, w1_d, w2_d, smf_d, b2r_d, ind_d, outT_d, b1_zero=True, b2_zero=True):
    from concourse.masks import make_identity

    f32 = mybir.dt.float32
    bf16 = mybir.dt.bfloat16
    ALU = mybir.AluOpType
    ACTF = mybir.ActivationFunctionType
    AXIS = mybir.AxisListType

    col_of = [0]
    for tg in CGS[:-1]:
        col_of.append(col_of[-1] + tg)

    with (
        tc.tile_pool(name="constp", bufs=1) as constp,
        tc.tile_pool(name="xp", bufs=1) as xp,
        tc.tile_pool(name="wp", bufs=1) as wp,
        tc.tile_pool(name="actp", bufs=3) as actp,
        tc.tile_pool(name="rp", bufs=2) as rp,
        tc.tile_pool(name="php", bufs=2, space="PSUM") as php,
        tc.tile_pool(name="pop", bufs=1, space="PSUM") as pop,
        tc.tile_pool(name="pmp", bufs=2, space="PSUM") as pmp,
    ):
        # ---- constants & input loads (split across 4 DMA queues so the
        # first mm1 can start ~3us in and nothing arrives late) ----
        identity = constp.tile([128, 128], f32, name="identity")
        make_identity(nc, identity)
        onesb = constp.tile([1, 512], bf16, name="onesb")
        nc.vector.memset(onesb, 1.0)

        # gpsimd (SWDGE) queue: small tensors -- separate issue pipe from the
        # HWDGE queue, so these land early without consuming HWDGE slots.
        # b1t+wgb packed in one tensor so a single early DMA covers both.
        smf = xp.tile([128, FC + HC * E], f32, name="smf", tag="smf")
        nc.gpsimd.dma_start(out=smf, in_=smf_d[:, :])
        b1t = smf[:, 0:FC]
        wgb = xp.tile([128, HC * E], bf16, name="wgb", tag="wgb")
        nc.vector.tensor_copy(out=wgb, in_=smf[:, FC:FC + HC * E])
        ind6 = xp.tile([8, 8 * 128], bf16, name="ind6", tag="ind6")
        nc.gpsimd.dma_start(out=ind6, in_=ind_d[:, :])
        b2r = None
        if not b2_zero:
            b2r = xp.tile([1, H], bf16, name="b2r", tag="b2r")
            nc.gpsimd.dma_start(out=b2r, in_=b2r_d[:, :])

        # All large loads on ONE queue, ordered by first-need time and split
        # into pieces sized to the PE's consumption rate.  DMA transfers
        # serialize at full bandwidth in hardware anyway (a single large copy
        # already spans all 16 engines), so a deliberate global order beats
        # spreading across queues.
        PIECES = [2, 2, 4, 4, 4]            # fc counts per w1/w2 piece
        pc_of = []                          # fc -> (piece, offset)
        for pi, nfc in enumerate(PIECES):
            pc_of += [(pi, k) for k in range(nfc)]
        xg = [None] * len(CGS)
        w1q = [None] * len(PIECES)
        w2q = [None] * len(PIECES)

        def load_xg(gi):
            tg = CGS[gi]
            xt = xp.tile([128, HC, tg], bf16, name=f"xs{gi}", tag=f"xs{gi}")
            nc.sync.dma_start(
                out=xt,
                in_=xT_d[:, col_of[gi]:col_of[gi] + tg].rearrange("(hc p) t -> p hc t", p=128))
            xg[gi] = xt

        def load_w1q(pi):
            f0 = sum(PIECES[:pi]) * 128
            nf = PIECES[pi] * 128
            wt = wp.tile([128, HC, nf], bf16, name=f"w1_{pi}", tag=f"w1_{pi}")
            nc.sync.dma_start(
                out=wt,
                in_=w1_d[:, f0:f0 + nf].rearrange("(hc p) f -> p hc f", p=128))
            w1q[pi] = wt

        def load_w2q(pi):
            f0 = sum(PIECES[:pi]) * 128
            wt = wp.tile([128, PIECES[pi], H], bf16, name=f"w2_{pi}", tag=f"w2_{pi}")
            nc.sync.dma_start(
                out=wt,
                in_=w2_d[f0:f0 + PIECES[pi] * 128, :].rearrange("(fc p) h -> p fc h", p=128))
            w2q[pi] = wt

        load_xg(0)
        load_w1q(0)
        load_w2q(0)
        load_w1q(1)
        load_w2q(1)
        for gi in range(1, len(CGS)):
            load_xg(gi)
        for pi in range(2, len(PIECES)):
            load_w1q(pi)
            load_w2q(pi)

        def w1sl(hc, fc):
            pi, k = pc_of[fc]
            return w1q[pi][:, hc, k * 128:k * 128 + 128]

        def w2sl(hc, fc):
            pi, k = pc_of[fc]
            return w2q[pi][:, k, hc * 128:(hc + 1) * 128]

        # ---- router (emitted mid-stream): scores -> local expert's
        # renormalized top-2 weight, w = sigmoid(2*s0 - m1 - m2).  Exact when
        # the local expert is in the device top-2 (invariant to which slot),
        # degrades smoothly on rank-boundary ties. ----
        comb_sb = []
        comb_bf = []
        chunks = []                          # (group, local col, n)
        for gi, tg in enumerate(CGS):
            for c in range(0, tg, 128):
                chunks.append((gi, c, min(128, tg - c)))
        wcols = xp.tile([128, len(chunks)], f32, name="wcols", tag="wcols")
        nc.vector.memset(wcols, 0.0)

        def emit_router_scores():
            # all sigmoids run BEFORE the first silu: Silu and Sigmoid live
            # in different ACT function tables, each swap costs 1.3us.  The
            # per-chunk max/mask chain alternates DVE / gpsimd so the last
            # sigmoid (which gates the Silu table load) lands early.
            for ci, (gi, c, n) in enumerate(chunks):
                eng = nc.vector
                tsl = slice(c, c + n)
                ps = pmp.tile([128, E], f32, name=f"ps{ci}", tag="pm")
                for hc in range(HC):
                    nc.tensor.matmul(
                        out=ps[0:n, :], lhsT=xg[gi][:, hc, tsl], rhs=wgb[:, hc * E:(hc + 1) * E],
                        start=(hc == 0), stop=(hc == HC - 1),
                    )
                s = rp.tile([128, E], f32, name=f"s{ci}", tag="s")
                eng.tensor_copy(out=s[0:n, :], in_=ps[0:n, :])
                m1 = rp.tile([128, 1], f32, name=f"m1{ci}", tag="m1")
                eng.tensor_reduce(out=m1[0:n, :], in_=s[0:n, :], axis=AXIS.X, op=ALU.max)
                is1 = rp.tile([128, E], f32, name=f"is1{ci}", tag="is1")
                eng.tensor_scalar(out=is1[0:n, :], in0=s[0:n, :], scalar1=m1[0:n, :], scalar2=None, op0=ALU.is_ge)
                s2 = rp.tile([128, E], f32, name=f"s2{ci}", tag="s2")
                eng.scalar_tensor_tensor(
                    out=s2[0:n, :], in0=is1[0:n, :], scalar=-1e30, in1=s[0:n, :], op0=ALU.mult, op1=ALU.add,
                )
                m2 = rp.tile([128, 1], f32, name=f"m2{ci}", tag="m2")
                eng.tensor_reduce(out=m2[0:n, :], in_=s2[0:n, :], axis=AXIS.X, op=ALU.max)
                # nm = -m1 - m2
                nm = rp.tile([128, 1], f32, name=f"nm{ci}", tag="nm")
                eng.scalar_tensor_tensor(
                    out=nm[0:n, :], in0=m1[0:n, :], scalar=-1.0, in1=m2[0:n, :], op0=ALU.mult, op1=ALU.subtract,
                )
                nc.scalar.activation(
                    out=wcols[0:n, ci:ci + 1], in_=s[0:n, 0:1], func=ACTF.Sigmoid,
                    bias=nm[0:n, :], scale=2.0,
                )

        pstsb = xp.tile([8, 128], bf16, name="pstsb", tag="pstsb")

        def emit_router_transposes():
            # ONE batched transpose of all w columns (PE transposes run
            # mid-stream where the PE is the gapless bottleneck, so each
            # saved pass is 1:1 on the makespan)
            nch = len(chunks)
            pst = pmp.tile([nch, 128], f32, name="pst", tag="pm")
            nc.tensor.transpose(out=pst, in_=wcols[:, :],
                                identity=identity[:, :])
            nc.vector.tensor_copy(out=pstsb[0:nch, :], in_=pst)

        def emit_router_combine():
            # broadcast chunk ci's w row across the 128 partitions with an
            # indicator matmul (lhsT row ci of ind6 is all-ones), writing
            # each chunk's column range of the group's combW.  The last
            # group also gets a bf16 copy: its activations are scaled on
            # the way INTO mm2, so its epilogue is a plain PSUM->SBUF copy.
            nch = len(chunks)
            cws = []
            for gi, tg in enumerate(CGS):
                cw = pmp.tile([128, tg], f32, name=f"combW{gi}", tag="pm")
                cws.append(cw)
            for ci, (gi, c, n) in enumerate(chunks):
                nc.tensor.matmul(
                    out=cws[gi][:, c:c + n],
                    lhsT=ind6[0:nch, ci * 128:ci * 128 + 128],
                    rhs=pstsb[0:nch, 0:n],
                    start=True, stop=True,
                )
            for gi, tg in enumerate(CGS):
                cs = xp.tile([128, tg], f32, name=f"combWs{gi}", tag=f"combWs{gi}")
                nc.vector.tensor_copy(out=cs, in_=cws[gi])
                comb_sb.append(cs)
                if gi == len(CGS) - 1:
                    cb = xp.tile([128, tg], bf16, name="combWb", tag="combWb")
                    nc.vector.tensor_copy(out=cb, in_=cws[gi])
                    comb_bf.append(cb)

        # ---- output accumulators (banks shared between the two groups) ----
        out_ps = []
        for gi, tg in enumerate(CGS):
            out_ps.append([pop.tile([128, tg], f32, name=f"outp{gi}_{hc}", tag=f"outp_{hc}")
                           for hc in range(HC)])

        # ---- main FFN loop; mm2 deferred two steps so PE never stalls on
        # the ACT silu chain or the early w2 DMA ----
        osb = xp.tile([128, HC, CAP], bf16, name="osb", tag="osb")
        steps = [(gi, fc) for gi in range(len(CGS)) for fc in range(FC)]
        DEFER = 3
        pending = []
        emit_router_scores()

        def emit_mm2(item):
            gi_p, fc_p, asil_p = item
            tg_p = CGS[gi_p]
            csl = slice(col_of[gi_p], col_of[gi_p] + tg_p)
            last_g = gi_p == len(CGS) - 1
            for hc in range(HC):
                nc.tensor.matmul(
                    out=out_ps[gi_p][hc], lhsT=w2sl(hc, fc_p),
                    rhs=asil_p, start=(fc_p == 0), stop=(fc_p == FC - 1 and b2_zero),
                )
                if fc_p == FC - 1:
                    if not b2_zero:
                        # b2 contribution as a rank-1 update closing the
                        # accum group, then scale by the combine weight;
                        # per-hc interleave so the tail drains per lane
                        nc.tensor.matmul(
                            out=out_ps[gi_p][hc], lhsT=b2r[:, hc * 128:(hc + 1) * 128],
                            rhs=onesb[:, 0:tg_p], start=False, stop=True,
                        )
                    if last_g and b2_zero:
                        # pre-scaled group: epilogue is a plain copy, split
                        # ACT / DVE so lanes drain in parallel (only these
                        # two engines can read PSUM)
                        if hc % 2 == 0:
                            nc.scalar.activation(
                                out=osb[:, hc, csl], in_=out_ps[gi_p][hc],
                                func=ACTF.Copy, bias=0.0, scale=1.0)
                        else:
                            nc.vector.tensor_copy(out=osb[:, hc, csl], in_=out_ps[gi_p][hc])
                    else:
                        # must be DVE: only DVE/ACT can read PSUM
                        nc.vector.tensor_mul(osb[:, hc, csl], out_ps[gi_p][hc], comb_sb[gi_p])
                    if last_g and hc % 2 == 1:
                        # store in hc pairs so the first transfer starts
                        # before the last lanes finish scaling
                        nc.sync.dma_start(
                            out=outT_d[(hc - 1) * 128:(hc + 1) * 128, csl].rearrange(
                                "(q p) t -> p q t", p=128),
                            in_=osb[:, hc - 1:hc + 1, csl])
            if fc_p == FC - 1 and not last_g:
                # store this group's columns (overlaps next group's compute)
                nc.sync.dma_start(
                    out=outT_d[:, csl].rearrange("(hc p) t -> p hc t", p=128),
                    in_=osb[:, :, csl])

        for si, (gi, fc) in enumerate(steps):
            tg = CGS[gi]
            hps = php.tile([128, tg], f32, name=f"h{gi}_{fc}", tag="h")
            for hc in range(HC):
                nc.tensor.matmul(
                    out=hps, lhsT=w1sl(hc, fc), rhs=xg[gi][:, hc, :],
                    start=(hc == 0), stop=(hc == HC - 1),
                )
            # Group 0 computes silu(h) as h * sigmoid(h): Sigmoid shares the
            # ACT function table with the router's sigmoids (Silu does not),
            # so the start of the kernel needs no table reload (1.3us each).
            # Group 1 switches to real Silu -- the one swap hides in ACT idle
            # mid-stream, and its shorter hps chain (no DVE hop) keeps the
            # PSUM hps buffers recycling fast in the final group.
            asil = actp.tile([128, tg], bf16, name=f"as{gi}_{fc}", tag="asil", bufs=DEFER + 2)
            if gi == 0 and b1_zero:
                sg = actp.tile([128, tg], bf16, name=f"sg{gi}_{fc}", tag="sg", bufs=DEFER + 2)
                nc.scalar.activation(
                    out=sg, in_=hps, func=ACTF.Sigmoid,
                    bias=b1t[:, fc:fc + 1], scale=1.0,
                )
                nc.vector.tensor_mul(asil, hps, sg)
            else:
                nc.scalar.activation(
                    out=asil, in_=hps, func=ACTF.Silu,
                    bias=b1t[:, fc:fc + 1], scale=1.0,
                )
            if gi == len(CGS) - 1 and b2_zero:
                # scale the last group's activations going INTO mm2 so its
                # epilogue needs no combine multiply
                asc = actp.tile([128, tg], bf16, name=f"ac{gi}_{fc}", tag="asc", bufs=DEFER + 2)
                nc.vector.tensor_mul(asc, asil, comb_bf[0])
                asil = asc
            pending.append((gi, fc, asil))
            if len(pending) > DEFER:
                emit_mm2(pending.pop(0))
            # router transposes/combine drop in once the sigmoids are done
            # (they sit in PE program order, so not too early)
            if si == 9:
                emit_router_transposes()
            elif si == 12:
                emit_router_combine()
        for item in pending:
            emit_mm2(item)


def _get_nc(b1_zero=True, b2_zero=True):
    key = ("nc", b1_zero, b2_zero)
    if key not in _cache:
        _cache[key] = _build_bass(b1_zero, b2_zero)
    return _cache[key]


def _route(x2d, Wg):
    """Host-side routing assignment (data placement only): token index lists
    per expert, matching the reference's fp32 top-2."""
    scores = x2d @ Wg.T                              # [T, E] fp32
    top1 = scores.argmax(1)
    s2 = scores.copy()
    s2[np.arange(scores.shape[0]), top1] = -np.inf
    top2 = s2.argmax(1)
    return [np.where((top1 == e) | (top2 == e))[0] for e in range(E)]


def _make_in_maps(x, Wg, W1, b1, W2, b2):
    x2d = np.ascontiguousarray(x.reshape(T, H), dtype=np.float32)
    idx = _route(x2d, Wg)
    xb = x2d.astype(ml_dtypes.bfloat16)
    in_maps = []
    for e in range(N_CORES):
        sel = idx[e][:CAP]
        n_e = len(sel)
        xT = np.zeros((H, CAP), ml_dtypes.bfloat16)
        xT[:, :n_e] = xb[sel].T
        perm = [e] + [i for i in range(E) if i != e]
        wgT = Wg[perm].T.astype(ml_dtypes.bfloat16)          # [H, E]
        wgb = wgT.reshape(HC, 128, E).transpose(1, 0, 2).reshape(128, HC * E)
        w1c = np.ascontiguousarray(W1[e]).astype(ml_dtypes.bfloat16)
        w2c = np.ascontiguousarray(W2[e]).astype(ml_dtypes.bfloat16)
        smf = np.empty((128, FC + HC * E), np.float32)
        smf[:, 0:FC] = b1[e].reshape(FC, 128).T
        smf[:, FC:] = wgb.astype(np.float32)
        b2rc = np.ascontiguousarray(b2[e].reshape(1, H)).astype(ml_dtypes.bfloat16)
        ind6 = np.kron(np.eye(8, dtype=np.float32),
                       np.ones((1, 128), np.float32)).astype(ml_dtypes.bfloat16)
        in_maps.append({"xT": xT, "w1": w1c, "w2": w2c, "smf": smf, "b2r": b2rc,
                        "ind6": ind6})
    return in_maps, idx


def _combine(outs, idx, x2d=None, Wg=None, W1=None, b1=None, W2=None, b2=None):
    """Scatter-add each expert's outputs back to token order."""
    of = np.zeros((T, H), np.float32)
    for e in range(N_CORES):
        sel = idx[e][:CAP]
        of[sel] += outs[e][:, :len(sel)].T.astype(np.float32)
        if len(idx[e]) > CAP and x2d is not None:
            # capacity overflow fallback (never hit for the staged shapes'
            # routing balance; exact fp32 math for generality)
            rest = idx[e][CAP:]
            sc = x2d[rest] @ Wg.T
            m1v = sc.max(1)
            s2v = sc.copy()
            s2v[np.arange(len(rest)), sc.argmax(1)] = -np.inf
            m2v = s2v.max(1)
            w = 1.0 / (1.0 + np.exp(-(2 * sc[:, e] - m1v - m2v)))
            hpre = x2d[rest] @ W1[e] + b1[e]
            a = hpre / (1.0 + np.exp(-hpre))
            of[rest] += w[:, None] * (a @ W2[e] + b2[e])
    return of.reshape(B, S, H)


def kernel(x, Wg, W1, b1, W2, b2, _trace=False, _trace_kwargs=None):
    from concourse.bass_utils import run_bass_kernel_spmd

    nc = _get_nc(b1_zero=bool(np.all(np.asarray(b1) == 0)),
                 b2_zero=bool(np.all(np.asarray(b2) == 0)))
    x = np.asarray(x, np.float32)
    Wg = np.asarray(Wg, np.float32)
    W1 = np.asarray(W1, np.float32)
    b1 = np.asarray(b1, np.float32)
    W2 = np.asarray(W2, np.float32)
    b2 = np.asarray(b2, np.float32)
    in_maps, idx = _make_in_maps(x, Wg, W1, b1, W2, b2)
    kw = {}
    if _trace:
        kw.update(trace=True, **(_trace_kwargs or {}))
    res = run_bass_kernel_spmd(nc, in_maps, core_ids=list(range(N_CORES)), **kw)
    _cache["last_results"] = res
    outs = [r["outT"] for r in res.results]
    return _combine(outs, idx, x.reshape(T, H), Wg, W1, b1, W2, b2)
